# revision 20
# baseline (speedup 1.0000x reference)
"""PointNet++ FP module kernel for 8 Trainium2 NeuronCores.

Problem: B=2, N=16384 query points, M=4096 known points.
  d2(q,m) -> 3-NN -> inverse-distance-weighted gather of known_feats ->
  concat with unknow_feats -> 2-layer 1x1-conv MLP with ReLU.

Sharding: queries are sharded 8 ways (4 slices per batch x 2 batches);
known points / features / weights are replicated per batch. Each core
handles 4096 queries independently; no collectives.

Per-core pipeline (32 tiles of 128 queries):
  PE    : S = 2*u.k - |k|^2 via K=4 matmul (fp32r), tile (128,4096) in PSUM
  ACT   : evict PSUM -> SBUF fp32
  DVE   : Max8 + MaxIndex -> top-3 scores + indices per query
  SWDGE : indirect DMA gather of 3x256-float feature rows per query
  DVE   : inverse-distance weights + weighted sum of gathered features
  PE    : transpose interp to channel-major, then MLP (fused over 4 tiles)
  ACT   : bias+ReLU evictions
"""

import json

import numpy as np

B, N, M = 2, 16384, 4096
C1, C2 = 128, 256
H1, H2 = 256, 128
NCORES = 8
NQ = N * B // NCORES          # queries per core = 4096
TILES = NQ // 128             # 32
GB_TILES = 4                  # tiles fused per MLP batch
SLICES = NCORES // B          # query slices per batch
CT = 320                      # combined gather row: 256 feats + 4 coords + pad
TK = 5                        # candidates rescored exactly per query


# ---------------------------------------------------------------------------
# BIR legalizer: this walrus build allows only ONE sem-wait per instruction.
# Split extra waits onto preceding single-wait NoOps on the same engine.
# ---------------------------------------------------------------------------
def _legalize_waits(bir_bytes: bytes) -> bytes:
    d = json.loads(bir_bytes)
    counter = [0]

    def fix_block(block):
        insts = block.get("instructions")
        if not isinstance(insts, list):
            return
        out = []
        for inst in insts:
            si = inst.get("sync_info") if isinstance(inst, dict) else None
            waits = si.get("on_wait") if si else None
            if waits and len(waits) > 1:
                for w in waits[:-1]:
                    counter[0] += 1
                    out.append({
                        "debug": inst.get("debug", 0),
                        "engine": inst["engine"],
                        "ins": [],
                        "name": f"I-wf{counter[0]}",
                        "opcode": "NoOp",
                        "outs": [],
                        "sync_info": {"on_update": [], "on_wait": [w]},
                    })
                si["on_wait"] = [waits[-1]]
            out.append(inst)
        block["instructions"] = out

    def walk(o):
        if isinstance(o, dict):
            if "instructions" in o:
                fix_block(o)
            for v in o.values():
                walk(v)
        elif isinstance(o, list):
            for v in o:
                walk(v)

    walk(d)
    return json.dumps(d).encode()


def _install_waitfix():
    import concourse.bass as bass
    if getattr(bass.Bass, "_waitfix_installed", False):
        return
    orig = bass.Bass.to_json_bytes

    def to_json_bytes(self):
        return _legalize_waits(orig(self))

    bass.Bass.to_json_bytes = to_json_bytes
    bass.Bass._waitfix_installed = True


# ---------------------------------------------------------------------------
# Bass program
# ---------------------------------------------------------------------------
_CACHE = {}


def _build_nc(variant=0):
    import concourse.bass as bass
    import concourse.mybir as mybir
    from concourse.tile import TileContext

    _install_waitfix()
    f32 = mybir.dt.float32
    f32r = mybir.dt.float32r
    u32 = mybir.dt.uint32

    nc = bass.Bass()
    uT_d = nc.dram_tensor("uT", [4, NQ], f32r, kind="ExternalInput")
    kT_d = nc.dram_tensor("kT", [4, M], f32r, kind="ExternalInput")
    u2_d = nc.dram_tensor("u2", [128, TILES], f32, kind="ExternalInput")
    uq_d = nc.dram_tensor("uq", [128, 4 * TILES], f32, kind="ExternalInput")
    uf_d = nc.dram_tensor("ufeat", [C1, NQ], f32r, kind="ExternalInput")
    # combined gather table: [256 feats | x y z |k|^2 | pad] = 320 f32 per row
    ctab_d = nc.dram_tensor("ctab", [M, CT], f32, kind="ExternalInput")
    w1_d = nc.dram_tensor("w1t", [128, 6 * 128], f32r, kind="ExternalInput")
    b1_d = nc.dram_tensor("b1c", [128, 2], f32, kind="ExternalInput")
    w2_d = nc.dram_tensor("w2t", [128, 2 * 128], f32r, kind="ExternalInput")
    b2_d = nc.dram_tensor("b2c", [128, 1], f32, kind="ExternalInput")
    id_d = nc.dram_tensor("ident", [128, 128], f32, kind="ExternalInput")
    out_d = nc.dram_tensor("out", [H2, NQ], f32, kind="ExternalOutput")

    AX = mybir.AxisListType
    OP = mybir.AluOpType
    AF = mybir.ActivationFunctionType

    with TileContext(nc) as tc:
        with (
            tc.tile_pool(name="res", bufs=1) as res,
            tc.tile_pool(name="s", bufs=3) as spool,
            tc.tile_pool(name="g", bufs=4) as gpool,
            tc.tile_pool(name="sm", bufs=4) as sm,
            tc.tile_pool(name="nf", bufs=2) as nfp,
            tc.tile_pool(name="outp", bufs=2) as outp,
            tc.tile_pool(name="ps", bufs=3, space="PSUM") as psp,
            tc.tile_pool(name="pst", bufs=2, space="PSUM") as pst,
            tc.tile_pool(name="psm", bufs=1, space="PSUM") as psm,
        ):
            # resident tensors
            uT = res.tile([4, NQ], f32r)
            nc.sync.dma_start(out=uT[:], in_=uT_d[:])
            kT = res.tile([4, M], f32r)
            nc.sync.dma_start(out=kT[:], in_=kT_d[:])
            u2 = res.tile([128, TILES], f32)
            nc.sync.dma_start(out=u2[:], in_=u2_d[:])
            uq = res.tile([128, 4 * TILES], f32)
            nc.sync.dma_start(out=uq[:], in_=uq_d[:])
            uf = res.tile([C1, NQ], f32r)
            nc.sync.dma_start(out=uf[:], in_=uf_d[:])
            w1 = res.tile([128, 6 * 128], f32r)
            nc.sync.dma_start(out=w1[:], in_=w1_d[:])
            b1 = res.tile([128, 2], f32)
            nc.sync.dma_start(out=b1[:], in_=b1_d[:])
            w2 = res.tile([128, 2 * 128], f32r)
            nc.sync.dma_start(out=w2[:], in_=w2_d[:])
            b2 = res.tile([128, 1], f32)
            nc.sync.dma_start(out=b2[:], in_=b2_d[:])
            ident = res.tile([128, 128], f32)
            nc.sync.dma_start(out=ident[:], in_=id_d[:])

            for gb in range(TILES // GB_TILES):
                nf0 = nfp.tile([128, GB_TILES * 128], f32r, tag="nf0")
                nf1 = nfp.tile([128, GB_TILES * 128], f32r, tag="nf1")
                for tl in range(GB_TILES):
                    t = gb * GB_TILES + tl
                    # --- scores S = 2 u.k - |k|^2 ---
                    s_sb = spool.tile([128, M], f32, tag="s_sb")
                    lhsT = uT[:, t * 128:(t + 1) * 128]
                    for c in range(M // 512):
                        ps = psp.tile([128, 512], f32, tag="s_ps")
                        nc.tensor.matmul(
                            ps[:], lhsT, kT[:, c * 512:(c + 1) * 512],
                            start=True, stop=True)
                        nc.scalar.activation(s_sb[:, c * 512:(c + 1) * 512], ps[:],
                                             AF.Copy)
                    # --- top-3 ---
                    m8 = sm.tile([128, 8], f32, tag="m8")
                    i8 = sm.tile([128, 8], u32, tag="i8")
                    if variant < 1:
                        nc.vector.max(out=m8[:], in_=s_sb[:])
                        nc.vector.max_index(out=i8[:], in_max=m8[:],
                                            in_values=s_sb[:])
                    else:
                        nc.vector.memset(m8[:], 1.0)
                        nc.vector.memset(i8[:], t)
                    # --- gather T combined rows (feats + coords) per query ---
                    g = gpool.tile([128, TK, CT], f32, tag="g")
                    if variant < 2:
                        for j in range(TK):
                            nc.gpsimd.indirect_dma_start(
                                out=g[:, j, :], out_offset=None, in_=ctab_d[:],
                                in_offset=bass.IndirectOffsetOnAxis(
                                    ap=i8[:, j:j + 1], axis=0))
                    else:
                        nc.vector.memset(g[:], 1.0)
                    # exact negated d2: nd = 2 u.k - |k|^2 - u2  (all fp32)
                    nd = sm.tile([128, 8], f32, tag="nd")
                    nc.vector.memset(nd[:], -1e30)
                    ndt = nd[:, 0:TK]
                    nc.vector.tensor_scalar_mul(
                        ndt, g[:, :, C2 + 0], uq[:, 4 * t + 0:4 * t + 1])
                    for dim in (1, 2):
                        nc.vector.scalar_tensor_tensor(
                            out=ndt, in0=g[:, :, C2 + dim],
                            scalar=uq[:, 4 * t + dim:4 * t + dim + 1],
                            in1=ndt, op0=OP.mult, op1=OP.add)
                    nc.vector.tensor_tensor(out=ndt, in0=ndt,
                                            in1=g[:, :, C2 + 3], op=OP.subtract)
                    nc.vector.tensor_scalar(
                        out=ndt, in0=ndt, scalar1=uq[:, 4 * t + 3:4 * t + 4],
                        scalar2=None, op0=OP.subtract)
                    # exact top-3 by rescored distance
                    m8b = sm.tile([128, 8], f32, tag="m8b")
                    nc.vector.max(out=m8b[:], in_=nd[:])
                    # inverse sqrt distances for all T candidates
                    dT = sm.tile([128, TK], f32, tag="dT")
                    nc.vector.tensor_scalar(
                        out=dT[:], in0=ndt, scalar1=-1.0, scalar2=0.0,
                        op0=OP.mult, op1=OP.max)
                    nc.scalar.activation(dT[:], dT[:], AF.Sqrt)
                    nc.vector.tensor_scalar(out=dT[:], in0=dT[:], scalar1=1e-8,
                                            scalar2=None, op0=OP.add)
                    rT = sm.tile([128, TK], f32, tag="rT")
                    nc.vector.reciprocal(rT[:], dT[:])
                    # mask: keep only the top-3 (nd >= 3rd-best)
                    mk = sm.tile([128, TK], f32, tag="mk")
                    nc.vector.tensor_scalar(
                        out=mk[:], in0=ndt, scalar1=m8b[:, 2:3],
                        scalar2=None, op0=OP.is_ge)
                    nc.vector.tensor_tensor(out=rT[:], in0=rT[:], in1=mk[:],
                                            op=OP.mult)
                    rs = sm.tile([128, 1], f32, tag="rs")
                    nc.vector.tensor_reduce(out=rs[:], in_=rT[:], axis=AX.X,
                                            op=OP.add)
                    nc.vector.reciprocal(rs[:], rs[:])
                    wT = sm.tile([128, TK], f32, tag="wT")
                    nc.vector.tensor_scalar_mul(wT[:], rT[:], rs[:, 0:1])
                    # --- weighted sum -> (128q, 256c) ---
                    acc = gpool.tile([128, C2], f32, tag="acc")
                    nc.vector.tensor_scalar_mul(
                        acc[:], g[:, 0, 0:C2], wT[:, 0:1])
                    for j in range(1, TK):
                        nc.vector.scalar_tensor_tensor(
                            out=acc[:], in0=g[:, j, 0:C2],
                            scalar=wT[:, j:j + 1],
                            in1=acc[:], op0=OP.mult, op1=OP.add)
                    # --- transpose to channel-major ---
                    for mblk in range(2):
                        tp = pst.tile([128, 128], f32, tag="tp")
                        nc.tensor.transpose(
                            tp[:], acc[:, mblk * 128:(mblk + 1) * 128], ident[:])
                        dst = nf0 if mblk == 0 else nf1
                        nc.scalar.activation(dst[:, tl * 128:(tl + 1) * 128],
                                             tp[:], AF.Copy)
                # --- MLP over the 4-tile batch ---
                nfree = GB_TILES * 128
                ufs = uf[:, gb * nfree:(gb + 1) * nfree]
                h_sb = outp.tile([128, 2, nfree], f32r, tag="h")
                for mblk in range(2):
                    hp = psm.tile([128, nfree], f32, tag="hp")
                    for k, rhs in enumerate((nf0[:], nf1[:], ufs)):
                        nc.tensor.matmul(
                            hp[:],
                            w1[:, (k * 2 + mblk) * 128:(k * 2 + mblk + 1) * 128]
                            ,
                            rhs,
                            start=(k == 0), stop=(k == 2))
                    nc.scalar.activation(h_sb[:, mblk, :], hp[:], AF.Relu,
                                         bias=b1[:, mblk:mblk + 1])
                op = psm.tile([128, nfree], f32, tag="op")
                for k in range(2):
                    nc.tensor.matmul(
                        op[:], w2[:, k * 128:(k + 1) * 128],
                        h_sb[:, k, :],
                        start=(k == 0), stop=(k == 1))
                o_sb = outp.tile([128, nfree], f32, tag="o")
                nc.scalar.activation(o_sb[:], op[:], AF.Relu, bias=b2[:, 0:1])
                nc.sync.dma_start(out=out_d[:, gb * nfree:(gb + 1) * nfree],
                                  in_=o_sb[:])

    import concourse.bass as bass  # noqa: F811
    return nc


def _prep_inputs(unknown, known, unknow_feats, known_feats, W1, b1, W2, b2):
    """Host-side prep: per-core input dicts."""
    f = np.float32
    unknown = np.asarray(unknown, f)
    known = np.asarray(known, f)
    unknow_feats = np.asarray(unknow_feats, f)
    known_feats = np.asarray(known_feats, f)
    W1 = np.asarray(W1, f); b1 = np.asarray(b1, f)
    W2 = np.asarray(W2, f); b2 = np.asarray(b2, f)

    w1t = np.ascontiguousarray(
        W1.T.reshape(3, 128, 2, 128).transpose(1, 0, 2, 3).reshape(128, 6 * 128))
    w2t = np.ascontiguousarray(
        W2.T.reshape(2, 128, 128).transpose(1, 0, 2).reshape(128, 2 * 128))
    b1c = np.ascontiguousarray(b1.reshape(2, 128).T)
    b2c = np.ascontiguousarray(b2.reshape(128, 1))
    ident = np.eye(128, dtype=f)

    maps = []
    for core in range(NCORES):
        b = core // SLICES
        s = core % SLICES
        usl = unknown[b, s * NQ:(s + 1) * NQ]          # (NQ, 3)
        ksl = known[b]                                  # (M, 3)
        uT = np.ascontiguousarray(
            np.stack([2 * usl[:, 0], 2 * usl[:, 1], 2 * usl[:, 2],
                      -np.ones(NQ, f)]))
        kT = np.ascontiguousarray(
            np.stack([ksl[:, 0], ksl[:, 1], ksl[:, 2],
                      (ksl * ksl).sum(-1)]))
        u2 = np.ascontiguousarray(
            (usl * usl).sum(-1).reshape(TILES, 128).T)
        # uq[p, 4t+d] = [2ux, 2uy, 2uz, |u|^2] for query q = t*128+p
        uqcols = np.concatenate([2 * usl, (usl * usl).sum(-1)[:, None]], 1)
        uq = np.ascontiguousarray(
            uqcols.reshape(TILES, 128, 4).transpose(1, 0, 2).reshape(128, 4 * TILES))
        ctab = np.zeros((M, CT), np.float32)
        ctab[:, 0:C2] = known_feats[b].T
        ctab[:, C2:C2 + 3] = ksl
        ctab[:, C2 + 3] = (ksl * ksl).sum(-1)
        ufeat = np.ascontiguousarray(unknow_feats[b][:, s * NQ:(s + 1) * NQ])
        maps.append({
            "uT": uT, "kT": kT, "u2": u2, "uq": uq, "ctab": ctab,
            "ufeat": ufeat,
            "w1t": w1t, "b1c": b1c, "w2t": w2t, "b2c": b2c, "ident": ident,
        })
    return maps


def run_cores(in_maps, **kwargs):
    from concourse.bass_utils import run_bass_kernel_spmd
    if "nc" not in _CACHE:
        _CACHE["nc"] = _build_nc()
    return run_bass_kernel_spmd(_CACHE["nc"], in_maps, list(range(NCORES)),
                                **kwargs)


def kernel(unknown, known, unknow_feats, known_feats, W1, b1, W2, b2):
    in_maps = _prep_inputs(unknown, known, unknow_feats, known_feats,
                           W1, b1, W2, b2)
    res = run_cores(in_maps)
    out = np.empty((B, H2, N), np.float32)
    for core in range(NCORES):
        b = core // SLICES
        s = core % SLICES
        out[b][:, s * NQ:(s + 1) * NQ] = res.results[core]["out"]
    return out


import concourse.bass as bass  # noqa: E402  (needed at trace time)
